# revision 10
# baseline (speedup 1.0000x reference)
"""Trainium2 Bass kernel for the MANTIS quantum-circuit-loss nn.Module. v2

Shapes (hardcoded): B=128, L=16, M=32, P=4.  8 NeuronCores, batch-sharded
(16 batch elements per core).

Math
----
Let j = (m, p) flattened (M*P = 128 == partition count) and
    A[b, l, j] = theta[l, j] + scal[p(j)] * input_ds[b, l]
    CA = cos(A), SA = sin(A)                       (ACT Sin + pi/2 bias)

prob term:      amp[b]  = sum_j coef_j prod_l CA[b,l,j]
normalization:  norm[b] = sum_{j,k} coef_j coef_k prod_l cos(A[b,l,j]-A[b,l,k])

Split the 16 sites into two groups of 8; per group build 256 branch-product
vectors U_g[j, T] by log-doubling (bf16).  With coef folded into U1:
    D_b[T1, T2] = sum_j (c U1)[j, T1] U2[j, T2]    (PE matmul, K = 128)
    norm[b] = sum_{T1,T2} D_b^2,  amp[b] = D_b[0, 0]
Host finishes: partition sums, ln, mean, tiny regularization variances.

v2 changes vs v1:
  - stage A / cos-sin / coef-fold split into an early (b0:4) pass so the
    doubling pipeline starts sooner.
  - amp extraction runs on Pool, off the DVE spine.
  - Q-mode DVE reduces from non-final chunks are deferred and flushed at
    the last chunk, so the DVE spine never stalls on ACT squares and the
    tail drains on both engines in parallel.
  - fin laid out as norm b0:12 | amps | norm b12:16 | H halves.
  - optional G (gram via DMA-XBAR transpose) mode and PE-warmup knobs
    exist but are off by default (measured slower / noise-level).
"""

import math
import os

import numpy as np

import concourse.bacc as bacc
import concourse.bass as bass
import concourse.bass_isa as bass_isa
import concourse.mybir as mybir
import concourse.tile as tile

B, L, M, P = 128, 16, 32, 4
NCORES = 8
BLOC = B // NCORES  # 16 batch elements per core
J = M * P  # 128
EPS = 1e-20
REG_C = 0.01
REG_THETA_M = 0.01
REG_THETA_P = 0.01

F32 = mybir.dt.float32
BF16 = mybir.dt.bfloat16
AF = mybir.ActivationFunctionType
ALU = mybir.AluOpType

# batch chunk sizes for the U-build / matmul / square pipeline
CHUNKS = [int(x) for x in os.environ.get("MANTIS_CHUNKS", "1,1,2,3,4,5").split(",")]
# per-b square-reduce mode (16 chars):
#   A = ACT Square + accum direct from PSUM (1 pass)
#   T = DVE tensor_tensor_reduce direct from PSUM (1 pass)
#   S = ACT Square -> SBUF bf16, Pool tensor_reduce XYZWC -> fin[0, i]
#   Q = ACT Square -> SBUF bf16, DVE sums (tensor_scalar accum)
#   W = DVE copy -> SBUF bf16, DVE squares+sums (STT accum)
#   H = column-split: ACT squares cols 0:256 (accum fin[:,i]), DVE TTR
#       cols 256:512 (accum fin[:, 32 + hslot], hslot assigned in order)
#   G = gram mode: DMA-transpose U1/U2, PE grams G1/G2, ACT copies G2 to
#       SBUF bf16, DVE STT (G1*G2) with accum.  Only whole chunks may be G.
SQ_MODES = os.environ.get("MANTIS_SQ", "AAQAAQAAQAAQQQAA")
# how many U chunks of group 1 go to Pool (from the END backward)
N_UPOOL = int(os.environ.get("MANTIS_UPOOL", "0"))
# per-chunk count of group-1 batches built on Pool (comma list, len == CHUNKS)
_upools = os.environ.get("MANTIS_UPOOLS", "")
UPOOLS = [int(x) for x in _upools.split(",")] if _upools else [0] * len(CHUNKS)
assert len(UPOOLS) == len(CHUNKS)
# doubling group-1 engine: pool | dve
G1_ENG = os.environ.get("MANTIS_G1ENG", "pool")
# doubling group-0 engine: dve | pool
G0_ENG = os.environ.get("MANTIS_G0ENG", "dve")
# amp extraction engine: pool frees the DVE spine
AMP_POOL = os.environ.get("MANTIS_AMP_POOL", "1") == "1"
# number of dummy PE matmuls emitted up front to ramp the PE p-state
N_PEWARM = int(os.environ.get("MANTIS_PEWARM", "0"))

# params column layout
PC_THETA = 0  # 16 cols: theta_t[j, l]
PC_COEF = 16  # 1 col
PC_SCAL = 17  # 1 col: pi / 2^(p(j)+1)
PC_HALFPI = 18  # 1 col: pi/2 (ACT bias for cos-via-sin)
P_COLS = 19

# fin layout: [J, 40]: 0:16 norm partials, 16:32 amp partials, 32:40 H halves
N_HSLOTS = 8
FIN_COLS = 32 + N_HSLOTS


def build_params() -> np.ndarray:
    pr = np.zeros((J, P_COLS), dtype=np.float32)
    sf = (np.pi / 2.0 ** (np.arange(P) + 1.0)).astype(np.float32)
    pr[:, PC_SCAL] = np.tile(sf, M)
    pr[:, PC_HALFPI] = np.pi / 2.0
    return pr


def build_program():
    """Build the SPMD Bass/Tile program (identical on all 8 cores)."""
    nc = bacc.Bacc(
        "TRN2",
        target_bir_lowering=False,
        debug=False,
        num_devices=NCORES,
    )
    params_d = nc.dram_tensor("params", [J, P_COLS], F32, kind="ExternalInput")
    inds_d = nc.dram_tensor("inds", [1, BLOC * L], mybir.dt.float32r, kind="ExternalInput")
    out_d = nc.dram_tensor("out", [J, FIN_COLS], F32, kind="ExternalOutput")

    with tile.TileContext(nc) as tc:
        with (
            tc.tile_pool(name="const", bufs=1) as cpool,
            tc.tile_pool(name="work", bufs=1) as wpool,
            tc.tile_pool(name="dps", bufs=7, space=bass.MemorySpace.PSUM) as dpool,
            tc.tile_pool(name="bps", bufs=1, space=bass.MemorySpace.PSUM) as bpool,
            tc.tile_pool(name="dsqp", bufs=8) as spool,
        ):
            _emit(nc, tc, cpool, wpool, dpool, bpool, spool, params_d, inds_d, out_d)
    nc.compile()
    return nc


def _emit(nc, tc, cpool, wpool, dpool, bpool, spool, params_d, inds_d, out_d):
    params = cpool.tile([J, P_COLS], F32, tag="params")

    # dummy Sin with no input deps: forces the trig_and_small ACT table
    # (sin + square) to load immediately, overlapped with the input DMA.
    scrd = wpool.tile([1, 2], F32, tag="scrd")
    nc.vector.memset(scrd[0:1, 0:1], 0.0)
    nc.scalar.activation(scrd[0:1, 1:2], scrd[0:1, 0:1], AF.Sin)

    inds_row = cpool.tile([1, BLOC * L], mybir.dt.float32r, tag="inds_row")
    nc.sync.dma_start(inds_row[:], inds_d[:, :])
    nc.sync.dma_start(params[:], params_d[:, :])

    # broadcast inds to all partitions via a K=1 f32r ones-matmul
    ones_row = wpool.tile([1, J], F32, tag="ones_row")
    nc.gpsimd.memset(ones_row[:], 1.0)
    inds_ps = bpool.tile([J, BLOC * L + 64], F32, tag="inds_ps")
    if N_PEWARM:
        # dummy matmuls with no input deps keep the PE busy from t~7us so
        # the p-state is ramped when the real matmuls arrive; they write a
        # scratch column range so they cannot interfere with the broadcast
        wrow = wpool.tile([1, 64], F32, tag="wrow")
        nc.vector.memset(wrow[:], 0.0)
        for _ in range(N_PEWARM):
            nc.tensor.matmul(
                inds_ps[:, BLOC * L : BLOC * L + 64],
                ones_row[:].bitcast(mybir.dt.float32r),
                wrow[:].bitcast(mybir.dt.float32r),
            )
    nc.tensor.matmul(
        inds_ps[:, 0 : BLOC * L],
        ones_row[:].bitcast(mybir.dt.float32r), inds_row[:],
    )

    theta_ap = params[:, PC_THETA : PC_THETA + L]
    coef_ap = params[:, PC_COEF : PC_COEF + 1]
    scal_ap = params[:, PC_SCAL : PC_SCAL + 1]

    # --- stage A: ARG[j, (i,l)] = theta[j,l] + scal[j]*inds[i,l]   (f32)
    # split into an early (b0:4) and late (b4:16) pass so the first L1
    # products can start ~1us sooner.
    arg = wpool.tile([J, BLOC * L], F32, tag="arg")
    in_bc = inds_ps[:, 0 : BLOC * L].rearrange("j (i l) -> j i l", i=BLOC, l=L)
    arg_v = arg[:].rearrange("j (i l) -> j i l", i=BLOC, l=L)
    cs = wpool.tile([J, 2 * BLOC * L], BF16, tag="cs")
    for (a0, a1) in ((0, 4), (4, BLOC)):
        th_bc = theta_ap.unsqueeze(1).broadcast_to([J, a1 - a0, L])
        nc.vector.scalar_tensor_tensor(
            out=arg_v[:, a0:a1], in0=in_bc[:, a0:a1], scalar=scal_ap,
            in1=th_bc, op0=ALU.mult, op1=ALU.add,
        )
        # CS[j, (t,i,l)]: t=0 -> cos(A) = sin(pi/2 - A), t=1 -> sin(A); bf16
        nc.scalar.activation(
            cs[:, a0 * L : a1 * L], arg[:, a0 * L : a1 * L], AF.Sin,
            bias=params[:, PC_HALFPI : PC_HALFPI + 1], scale=-1.0,
        )
        nc.scalar.activation(
            cs[:, BLOC * L + a0 * L : BLOC * L + a1 * L],
            arg[:, a0 * L : a1 * L], AF.Sin,
        )

    # fold coef into site l=0 (both branches) => every T1 combo of group 0
    # carries exactly one coef_j factor.
    cs_v = cs[:].rearrange("j (t i l) -> j t i l", t=2, i=BLOC, l=L)
    # coef is folded into the site-0 factor inside the L1 products (STT)

    # fin1: norm partials b0..11 (cols 0:12) + amp partials (cols 12:28);
    # fin2: norm partials b12..15 (cols 0:4) + H halves (cols 4:12).
    # Split so the bulk of the output DMA overlaps the tail of compute.
    fin_t = wpool.tile([J, 40], F32, tag="fin")
    fin = fin_t[:]
    nc.gpsimd.memset(fin, 0.0)
    fin1 = fin[:, 0:28]
    fin2 = fin[:, 28:40]

    def norm_acc(i):
        return fin1[:, i : i + 1] if i < 12 else fin2[:, i - 12 : i - 11]

    # --- doubling: L1 (site pairs, 4 combos), L2 (quads, 16 combos)
    # group 0 on DVE, group 1 on Pool (independent chains).
    eng = {
        0: nc.gpsimd if G0_ENG == "pool" else nc.vector,
        1: nc.gpsimd if G1_ENG == "pool" else nc.vector,
    }
    l1 = [wpool.tile([J, BLOC * 16], BF16, tag=f"l1_{g}", name=f"l1_{g}") for g in range(2)]
    l2 = [wpool.tile([J, BLOC * 32], BF16, tag=f"l2_{g}", name=f"l2_{g}") for g in range(2)]
    # two passes: batches 0:4 first so early chunk matmuls and squares
    # start earlier; 4:16 follows while the pipeline drains.
    for (b0, b1) in ((0, 4), (4, BLOC)):
        nb = b1 - b0
        for g in range(2):
            lo = g * 8  # first site of the group
            # L1[j, i, s, t1, t2] = CS[j,t1,i,lo+2s] * CS[j,t2,i,lo+2s+1]
            o1all = l1[g][:].rearrange(
                "j (i s t1 t2) -> j i s t1 t2", i=BLOC, s=4, t1=2, t2=2
            )
            for t1 in range(2):
                if g == 0:
                    # site-0 pair carries the coef factor: (CS0*coef)*CS1
                    in1 = (
                        cs_v[:, t1, b0:b1, 0:1]
                        .unsqueeze(3)
                        .broadcast_to([J, nb, 1, 2])
                    )
                    in2 = cs_v[:, :, b0:b1, 1:2].transpose([0, 2, 3, 1])
                    eng[g].scalar_tensor_tensor(
                        out=o1all[:, b0:b1, 0:1, t1, :], in0=in1,
                        scalar=coef_ap, in1=in2, op0=ALU.mult, op1=ALU.mult,
                    )
                    in1 = (
                        cs_v[:, t1, b0:b1, 2:8:2]
                        .unsqueeze(3)
                        .broadcast_to([J, nb, 3, 2])
                    )
                    in2 = cs_v[:, :, b0:b1, 3:8:2].transpose([0, 2, 3, 1])
                    eng[g].tensor_tensor(
                        out=o1all[:, b0:b1, 1:4, t1, :], in0=in1, in1=in2,
                        op=ALU.mult,
                    )
                    continue
                in1 = (
                    cs_v[:, t1, b0:b1, lo : lo + 8 : 2]
                    .unsqueeze(3)
                    .broadcast_to([J, nb, 4, 2])
                )
                in2 = cs_v[:, :, b0:b1, lo + 1 : lo + 8 : 2].transpose([0, 2, 3, 1])
                o1 = o1all[:, b0:b1, :, t1, :]
                eng[g].tensor_tensor(out=o1, in0=in1, in1=in2, op=ALU.mult)
            # L2[j, i, d, q1, q2] = L1[j,i,2d,q1] * L1[j,i,2d+1,q2]
            l1v = l1[g][:].rearrange("j (i s c) -> j i s c", i=BLOC, s=4, c=4)
            o2all = l2[g][:].rearrange(
                "j (i d q1 q2) -> j i d q1 q2", i=BLOC, d=2, q1=4, q2=4
            )
            for d in range(2):
                in1 = l1v[:, b0:b1, 2 * d, :].unsqueeze(3).broadcast_to([J, nb, 4, 4])
                in2 = l1v[:, b0:b1, 2 * d + 1, :].unsqueeze(2).broadcast_to([J, nb, 4, 4])
                o2 = o2all[:, b0:b1, d, :, :]
                eng[g].tensor_tensor(out=o2, in0=in1, in1=in2, op=ALU.mult)

    # --- L3 chunked by batch; per-chunk U tiles so PE/consumers pipeline
    sq_modes = SQ_MODES
    assert len(sq_modes) == BLOC and set(sq_modes) <= set("ASQWHRG")
    deferred = []  # (dsq, acc) DVE reduces emitted after the last chunk
    hslot = 0
    i0 = 0
    for c, csz in enumerate(CHUNKS):
        cw = csz * 256
        uc = [
            wpool.tile([J, cw], BF16, tag=f"u_{g}_{c}", name=f"u_{g}_{c}")
            for g in range(2)
        ]
        for g in range(2):
            l2v = l2[g][:].rearrange(
                "j (i d c16) -> j i d c16", i=BLOC, d=2, c16=16
            )
            ov = uc[g][:].rearrange(
                "j (i u1 u2) -> j i u1 u2", i=csz, u1=16, u2=16
            )
            # group 1 may be split between Pool (first kp batches) and DVE
            kp = min(UPOOLS[c], csz) if g == 1 else 0
            if g == 1 and c >= len(CHUNKS) - N_UPOOL:
                kp = csz
            if kp == 0 and c == len(CHUNKS) - 1 and csz >= 4:
                # split the last chunk's U so the first batches' matmuls and
                # squares start while the rest of the U is still building
                pieces = [(0, 2, nc.vector), (2, csz, nc.vector)]
            else:
                pieces = [(0, kp, nc.gpsimd), (kp, csz, nc.vector)]
            for (ka, kb, ueng) in pieces:
                if kb <= ka:
                    continue
                in1 = (
                    l2v[:, i0 + ka : i0 + kb, 0, :]
                    .unsqueeze(3)
                    .broadcast_to([J, kb - ka, 16, 16])
                )
                in2 = (
                    l2v[:, i0 + ka : i0 + kb, 1, :]
                    .unsqueeze(2)
                    .broadcast_to([J, kb - ka, 16, 16])
                )
                ueng.tensor_tensor(
                    out=ov[:, ka:kb, :, :], in0=in1, in1=in2, op=ALU.mult
                )

        # amp partials for this chunk (Pool; cheap and off the DVE spine):
        # fin[:, 16+i] = cU1[j,i,0]*U2[j,i,0]
        u1v = uc[0][:].rearrange("j (i t) -> j i t", i=csz, t=256)
        u2v = uc[1][:].rearrange("j (i t) -> j i t", i=csz, t=256)
        amp_eng = nc.gpsimd if AMP_POOL else nc.vector
        amp_eng.tensor_tensor(
            out=fin1[:, 12 + i0 : 12 + i0 + csz],
            in0=u1v[:, :, 0], in1=u2v[:, :, 0], op=ALU.mult,
        )
        if c == len(CHUNKS) - 1:
            # drain earlier chunks' deferred DVE reduces while ACT squares
            # the final chunk
            for (kind, dsq_d, acc_d) in deferred:
                if kind == "R":
                    nc.vector.tensor_reduce(
                        out=acc_d, in_=dsq_d[:], axis=mybir.AxisListType.X,
                        op=ALU.add,
                    )
                else:
                    nc.vector.tensor_scalar(
                        out=dsq_d[:], in0=dsq_d[:], scalar1=1.0, scalar2=None,
                        op0=ALU.mult, op1=ALU.add, accum_out=acc_d,
                    )
            deferred.clear()


        # G-chunks: batch-transpose U1/U2 once per chunk, then per-b grams
        is_g = all(sq_modes[i0 + k] == "G" for k in range(csz))
        assert is_g or not any(sq_modes[i0 + k] == "G" for k in range(csz)), (
            "G mode must cover whole chunks"
        )
        ut = None
        if is_g:
            ut = [
                spool.tile([J, cw], BF16, tag=f"ut{g}", name=f"ut{g}_{c}")
                for g in range(2)
            ]
            for g in range(2):
                utv = ut[g][:].rearrange("p (m j) -> p m j", m=2 * csz, j=J)
                nc.sync.dma_start_transpose(utv, uc[g][:])

        # D matmuls + square-reduce for this chunk's batch elements
        for k in range(csz):
            i = i0 + k
            dt = dpool.tile([J, 512], F32, tag="D")
            if is_g:
                # grams: G1 = (cU1)(cU1)^T in cols 0:128, G2 = U2 U2^T in 128:256
                for g in range(2):
                    o = dt[:, g * 128 : (g + 1) * 128]
                    a = ut[g][:, k * 256 : k * 256 + 128]
                    b = ut[g][:, k * 256 + 128 : (k + 1) * 256]
                    nc.tensor.matmul(o, a, a, start=True, stop=False)
                    nc.tensor.matmul(o, b, b, start=False, stop=True)
                g2s = spool.tile([J, J], BF16, tag="g2s", name=f"g2s_{i}")
                nc.scalar.activation(g2s[:], dt[:, 128:256], AF.Copy)
                gsq = spool.tile([J, J], BF16, tag="gsq", name=f"gsq_{i}")
                nc.vector.scalar_tensor_tensor(
                    out=gsq[:], in0=dt[:, 0:128], scalar=1.0, in1=g2s[:],
                    op0=ALU.mult, op1=ALU.mult, accum_out=norm_acc(i),
                )
                continue
            rhs = uc[1][:, k * 256 : (k + 1) * 256]
            for h in range(2):
                lhsT = uc[0][:, k * 256 + h * 128 : k * 256 + (h + 1) * 128]
                nc.tensor.matmul(dt[:, h * 256 : (h + 1) * 256], lhsT, rhs)
            mode = sq_modes[i]
            acc = norm_acc(i)
            if mode == "A":
                # 1-pass: ACT square + accum straight from PSUM
                nc.scalar.activation(dt[:], dt[:], AF.Square, accum_out=acc)
            elif mode == "R":
                # ACT square -> SBUF bf16, DVE tensor_reduce (2x bf16)
                dsq = spool.tile([J, 512], BF16, tag="dsq", name=f"dsq_{i}")
                nc.scalar.activation(dsq[:], dt[:], AF.Square)
                if c < len(CHUNKS) - 1:
                    deferred.append(("R", dsq, acc))
                else:
                    nc.vector.tensor_reduce(
                        out=acc, in_=dsq[:], axis=mybir.AxisListType.X,
                        op=ALU.add,
                    )
            elif mode in ("S", "Q"):
                dsq = spool.tile([J, 512], BF16, tag="dsq", name=f"dsq_{i}")
                nc.scalar.activation(dsq[:], dt[:], AF.Square)
                if mode == "S":
                    # Pool full reduce (partitions+cols) -> partition 0 only
                    facc = norm_acc(i)
                    nc.gpsimd.tensor_reduce(
                        out=facc[0:1, :], in_=dsq[:],
                        axis=mybir.AxisListType.XYZWC, op=ALU.add,
                    )
                elif c < len(CHUNKS) - 1:
                    # defer the DVE reduce so the DVE spine never stalls
                    # waiting for this ACT square; flushed at the last chunk
                    deferred.append(("Q", dsq, acc))
                else:
                    nc.vector.tensor_scalar(
                        out=dsq[:], in0=dsq[:], scalar1=1.0, scalar2=None,
                        op0=ALU.mult, op1=ALU.add, accum_out=acc,
                    )
            elif mode == "H":  # split: ACT cols 0:256, DVE STT cols 256:512
                assert hslot < N_HSLOTS
                nc.scalar.activation(
                    dt[:, 0:256], dt[:, 0:256], AF.Square, accum_out=acc
                )
                dsq = spool.tile([J, 256], BF16, tag="dsqh", name=f"dsqh_{i}")
                nc.vector.tensor_copy(dsq[:], dt[:, 256:512])
                nc.vector.scalar_tensor_tensor(
                    out=dsq[:], in0=dsq[:], scalar=1.0, in1=dsq[:],
                    op0=ALU.mult, op1=ALU.mult,
                    accum_out=fin2[:, 4 + hslot : 5 + hslot],
                )
                hslot += 1
            else:  # W: DVE copies, then squares+sums on DVE
                dsq = spool.tile([J, 512], BF16, tag="dsq", name=f"dsq_{i}")
                nc.vector.tensor_copy(dsq[:], dt[:])
                nc.vector.scalar_tensor_tensor(
                    out=dsq[:], in0=dsq[:], scalar=1.0, in1=dsq[:],
                    op0=ALU.mult, op1=ALU.mult, accum_out=acc,
                )
        i0 += csz
    assert i0 == BLOC

    assert not deferred

    nc.sync.dma_start(out_d[:, :], fin)


def make_in_maps(input_ds, theta, coef):
    input_ds = np.asarray(input_ds, dtype=np.float32)
    theta = np.asarray(theta, dtype=np.float32)
    coef = np.asarray(coef, dtype=np.float32)
    pr = build_params()
    pr[:, PC_THETA : PC_THETA + L] = theta.transpose(1, 2, 0).reshape(J, L)
    pr[:, PC_COEF] = coef.reshape(J)
    in_maps = []
    for c in range(NCORES):
        sl = np.ascontiguousarray(
            input_ds[c * BLOC : (c + 1) * BLOC, :].reshape(1, BLOC * L)
        )
        in_maps.append({"params": pr, "inds": sl})
    return in_maps


_NC_CACHE = None


def _get_program():
    global _NC_CACHE
    if _NC_CACHE is None:
        _NC_CACHE = build_program()
    return _NC_CACHE


def combine_outputs(results, theta, coef):
    """Host-side tail: partition sums, ln, mean, and regularization."""
    theta = np.asarray(theta, dtype=np.float32)
    coef = np.asarray(coef, dtype=np.float32)
    # map H slots back to their batch indices
    h_idx = [i for i in range(BLOC) if SQ_MODES[i] == "H"]
    ln_sum = 0.0
    for c in range(NCORES):
        fin = np.asarray(results[c]["out"], dtype=np.float64)  # [J, 40]
        norm = np.concatenate([fin[:, 0:12], fin[:, 28:32]], axis=1).sum(axis=0)
        for s, i in enumerate(h_idx):
            norm[i] += fin[:, 32 + s].sum()
        amp = fin[:, 12:28].sum(axis=0)  # [16]
        prob = amp * amp
        ln_sum += float(np.sum(np.log(prob + EPS * norm) - np.log(norm)))
    loss = -ln_sum / float(B)
    tf = theta.astype(np.float64)
    cf = coef.astype(np.float64)
    loss += REG_THETA_M * float(np.mean(np.var(tf, axis=1, ddof=1)))
    loss += REG_THETA_P * float(np.mean(np.var(tf, axis=2, ddof=1)))
    loss += REG_C * float(np.var(cf, ddof=1))
    return np.float32(loss)


def kernel(input_ds, theta, coef):
    from concourse.bass_utils import run_bass_kernel_spmd

    nc = _get_program()
    in_maps = make_in_maps(input_ds, theta, coef)
    res = run_bass_kernel_spmd(nc, in_maps, core_ids=list(range(NCORES)))
    return combine_outputs(res.results, theta, coef)
